# revision 44
# baseline (speedup 1.0000x reference)
"""AnchorAttention Trainium2 kernel, SPMD over 8 NeuronCores — head-split.

Sharding: core i -> (batch b = i//2, head half j = i%2).  Each core
processes ALL 4096 tokens of its batch for its 8 heads: QKV/Q
column-parallel, proj row-parallel; the host sums the two proj partials
per batch and adds bproj.

v2 changes vs v1 (263µs baseline):
  - Scores for each (a-tile, head-pair) land in ONE [128,1024] f32 PSUM
    tile ([pse|pso], 2 banks); a single Exp ACTIVATE covers both heads,
    halving ScalarE instruction count (the v1 steady-state pacer).
  - AV even/odd accumulate into one [128,1024] PSUM tile; one
    reciprocal over [64,1024] covers both denominators.
  - PSUM laid out by tag: ps2 2x2 banks, av 1x2, mm 2x1 = 8 banks, with
    the per-step PE issue order arranged so score-bank reuse never
    waits on the Exp that frees it.
  - All DMA moved to the sync-engine HWDGE queue (gpsimd issues
    nothing), batched: one dma_start per x block [128,8x512], weights
    in halves, y stores as [128,1024] pairs.  This removes v1's 10.8µs
    software-DGE drain + late y-issue serialization in the tail.
"""

import sys
from contextlib import ExitStack
from functools import partial

sys.path.insert(0, "/opt/trn_rl_repo")

import ml_dtypes
import numpy as np

import concourse.bass as bass
import concourse.mybir as mybir
import concourse.tile as tile
from concourse import bacc
from concourse.bass_utils import run_bass_kernel_spmd

F32 = mybir.dt.float32
BF16 = mybir.dt.bfloat16

B, S, DIM = 4, 4096, 1024
H, D = 16, 64
A = 512              # anchor tokens
TOK = 4096           # tokens per core (full batch)
NBLK = 8             # 512-token blocks per core
BLK = 512
N_CORES = 8
SCALE = 1.0 / np.sqrt(D)

HD = DIM // 2        # qk/v dims per core (8 heads x 64)
KQ = HD // 128       # 4 qk-dim tiles per core
KD = DIM // 128      # 8 contraction tiles (x width)
NA = A // 128        # 4 anchor tiles
NP = H // 4          # 4 head pairs per core

_COMPILED = {}


def build_kernel():
    nc = bacc.Bacc(trn_type="TRN2", target_bir_lowering=False)

    # All inputs arrive PRE-TILED by the host so each [128, 4096] SBUF tile
    # is 128 contiguous 8KB DRAM runs — one dma_start per tensor at full
    # descriptor efficiency.  xT row block blk holds x-block blk with layout
    # [p, 512*k + t] = x[512*blk + t, 128*k + p]; weights [p, 512*k + c].
    xT = nc.declare_dram_parameter("xT", [NBLK * 128, KD * BLK], BF16, isOutput=False)
    wk = nc.declare_dram_parameter("wk", [128, KD * BLK], BF16, isOutput=False)
    wv = nc.declare_dram_parameter("wv", [128, KD * BLK], BF16, isOutput=False)
    wqa = nc.declare_dram_parameter("wqa", [128, KD * BLK], BF16, isOutput=False)
    wqb = nc.declare_dram_parameter("wqb", [128, KD * BLK], BF16, isOutput=False)
    wproj = nc.declare_dram_parameter("wproj", [128, KQ * DIM], BF16, isOutput=False)
    y = nc.declare_dram_parameter("y", [TOK, DIM], BF16, isOutput=True)

    with tile.TileContext(nc) as tc, ExitStack() as ctx:
        # one SBUF pool for everything (per-tag bufs) — fewer pools means
        # fewer teardown drains / semaphore resets in the measured epilogue
        sb = ctx.enter_context(tc.tile_pool(name="sb", bufs=1))
        const = p_w = p_kt = p_v = sb
        p_xt = p_qt = p_exp = p_rb = p_ot = p_y = sb
        # PSUM: ps2 2x[128,1024] (4 banks) + av 1x[128,1024] (2 banks)
        #       + mm 2x[128,512] (2 banks) = 8 banks exactly.
        p_ps = ctx.enter_context(tc.tile_pool(name="p_ps", bufs=1, space="PSUM"))

        def ps2_tile(name):
            return p_ps.tile([128, 1024], F32, tag="ps2", bufs=2, name=name)

        def av_tile(name):
            return p_ps.tile([128, 1024], F32, tag="av", bufs=1, name=name)

        def mm_tile(name):
            return p_ps.tile([128, BLK], F32, tag="mm", bufs=2, name=name)

        # ---- PE warm-up while the first DMAs land.  Sized to end right as
        # xt0/wk finish landing: too long delays KT, too short lets the HAM
        # MID window re-throttle the PE before KT. ----
        warm = const.tile([128, 512], BF16, tag="warm")
        nc.vector.memset(warm[:], 0.0)
        wps = av_tile("warmps")
        NWARM = 16
        for i in range(NWARM):
            nc.tensor.matmul(
                wps[:, 0:512], warm[:, 0:128], warm[:],
                start=(i == 0), stop=(i == NWARM - 1),
            )

        # ---- input DMAs: one issue per tensor (128 x 8KB descriptors).
        # The anchor tokens ARE the first x block, so xt0 doubles as aT. ----
        xt_sb = [None] * NBLK

        def issue_xt(blk, eng):
            t = p_xt.tile([128, KD * BLK], BF16, tag="xt", bufs=3, name=f"xt{blk}")
            eng.dma_start(t[:], xT[128 * blk : 128 * (blk + 1), :])
            xt_sb[blk] = t

        def load_w(dram, name, eng):
            t = p_w.tile([128, dram.shape[1]], BF16, tag=name, name=name)
            eng.dma_start(t[:], dram[:, :])
            return t

        # xt0+wk are the KT-critical 2MB: split each across BOTH queues so
        # their first halves land ~5µs before the second halves; KT runs in
        # two k-half passes paced to that.
        t0 = p_xt.tile([128, KD * BLK], BF16, tag="xt", bufs=3, name="xt0")
        HB = KD * BLK // 2
        nc.sync.dma_start(t0[:, 0:HB], xT[0:128, 0:HB])
        wk_sb = p_w.tile([128, KD * BLK], BF16, tag="wk", name="wk_t")
        nc.scalar.dma_start(wk_sb[:, 0:HB], wk[:, 0:HB])
        nc.scalar.dma_start(t0[:, HB:], xT[0:128, HB:])
        nc.sync.dma_start(wk_sb[:, HB:], wk[:, HB:])
        xt_sb[0] = t0
        wqa_sb = load_w(wqa, "wqa", nc.scalar)
        wv_sb = load_w(wv, "wv", nc.sync)
        issue_xt(1, nc.sync)
        wqb_sb = load_w(wqb, "wqb", nc.scalar)
        wp_sb = load_w(wproj, "wp", nc.scalar)               # [128, 4*1024]

        def xt_c(blk, k):  # [128,512] chunk k of block blk
            return xt_sb[blk][:, BLK * k : BLK * (k + 1)]

        def w_c(t, k):     # [128,512] chunk k of a packed qkv weight
            return t[:, BLK * k : BLK * (k + 1)]

        # ---- KT[qk, a] = Wk^T aT, in two k-half passes paced to the split
        # DMAs (first pass starts as soon as the first halves land) ----
        ktps = [ps2_tile(f"ktps{h}") for h in range(2)]
        for half in range(2):
            for m in range(KQ):
                for k in range(KD // 2 * half, KD // 2 * (half + 1)):
                    nc.tensor.matmul(
                        ktps[m // 2][:, 512 * (m % 2) : 512 * (m % 2) + 512],
                        w_c(wk_sb, k)[:, 128 * m : 128 * (m + 1)],
                        xt_c(0, k),
                        start=(k == 0), stop=(k == KD - 1),
                    )
            if half == 0:
                # cushion: keep the PE busy while the second DMA halves
                # land, so the HAM stays at full clock for KT-B/V
                for i in range(4):
                    nc.tensor.matmul(
                        wps[:, 0:512], warm[:, 0:128], warm[:],
                        start=(i == 0), stop=(i == 3),
                    )
        # packed kt: kt_sb[h] = [kt(2h) | kt(2h+1)], each [128, A]
        kt_sb = []
        for h in range(2):
            kt = p_kt.tile([128, 2 * A], BF16, name=f"kt{h}", tag=f"kt{h}")
            nc.vector.tensor_copy(kt[:], ktps[h][:])
            kt_sb.append(kt)

        def kt_c(i):       # [128, A] view for head pair i
            return kt_sb[i // 2][:, A * (i % 2) : A * (i % 2) + A]

        # ---- V (a-major), then packed [ones|V_even|ones|V_odd] tiles ----
        for i in range(2):  # cushion over the wv-DMA wait
            nc.tensor.matmul(
                wps[:, 0:512], warm[:, 0:128], warm[:], start=(i == 0), stop=(i == 1)
            )
        # V part 1 (a=0,1) stays in the front; part 2 (a=2,3) is deferred
        # into block-0 step 0, which is Exp-bound with PE slack — it only
        # must complete before AV(0,0) at step (0,1).  It accumulates into
        # the av-tag PSUM tile (idle until AV(0,0)) to avoid ps2 conflicts.
        def emit_v_mms(dst, a_pair):
            for j, a in enumerate(a_pair):
                for k in range(KD):
                    nc.tensor.matmul(
                        dst[:, 512 * j : 512 * j + 512],
                        xt_c(0, k)[:, 128 * a : 128 * (a + 1)],
                        w_c(wv_sb, k),
                        start=(k == 0), stop=(k == KD - 1),
                    )

        def emit_v_pack(src, a_pair):
            for j, a in enumerate(a_pair):
                vr = v_sb[a][:].rearrange("p (hp c) -> p hp c", c=4 * D)
                pr = src[:, 512 * j : 512 * j + 512].rearrange(
                    "p (hp c) -> p hp c", c=2 * D
                )
                nc.vector.tensor_copy(vr[:, :, D : 2 * D], pr[:, :, 0:D])
                nc.vector.tensor_copy(vr[:, :, 3 * D : 4 * D], pr[:, :, D : 2 * D])

        vps0 = ps2_tile("vps0")
        emit_v_mms(vps0, (0, 1))
        v_sb = []
        for a in range(NA):
            t = p_v.tile([128, 2 * HD], BF16, name=f"v{a}", tag=f"v{a}")
            nc.vector.memset(
                t[:].rearrange("p (hp c) -> p hp c", c=2 * D)[:, :, 0:D], 1.0
            )
            v_sb.append(t)
        emit_v_pack(vps0, (0, 1))

        def emit_v_part2():
            v2ps = av_tile("v2ps")
            emit_v_mms(v2ps, (2, 3))
            emit_v_pack(v2ps, (2, 3))

        # ---- Q-proj of block 0 (anchors: wqa), m-major ----
        qt_sb = [[None] * KQ for _ in range(NBLK)]
        qps = [ps2_tile(f"q0ps{h}") for h in range(2)]
        for m in range(KQ):
            for k in range(KD):
                nc.tensor.matmul(
                    qps[m // 2][:, 512 * (m % 2) : 512 * (m % 2) + 512],
                    w_c(wqa_sb, k)[:, 128 * m : 128 * (m + 1)],
                    xt_c(0, k),
                    start=(k == 0), stop=(k == KD - 1),
                )
        for m in range(KQ):
            qt = p_qt.tile([128, BLK], BF16, tag="qt", bufs=8, name=f"q0_{m}")
            nc.vector.tensor_copy(
                qt[:], qps[m // 2][:, 512 * (m % 2) : 512 * (m % 2) + 512]
            )
            qt_sb[0][m] = qt

        ot_sb = [[None] * KQ for _ in range(NBLK)]

        def emit_score_tile(blk, i, a):
            """One [128,1024] PSUM tile = [pse(a)|pso(a)] for head pair i;
            one Exp ACTIVATE over both halves -> e tile [128,1024] bf16."""
            ps = ps2_tile(f"sc{blk}_{i}_{a}")
            nc.tensor.matmul(
                ps[:, 0:512],
                kt_c(i)[0:D, 128 * a : 128 * (a + 1)],
                qt_sb[blk][i][0:D, :],
                start=True, stop=True,
                tile_position=(0, 0),
            )
            nc.tensor.matmul(
                ps[:, 512:1024],
                kt_c(i)[D : 2 * D, 128 * a : 128 * (a + 1)],
                qt_sb[blk][i][D : 2 * D, :],
                start=True, stop=True,
                tile_position=(64, 0),
            )
            e = p_exp.tile([128, 1024], BF16, tag="exp", bufs=12)
            nc.scalar.activation(
                e[:], ps[:], mybir.ActivationFunctionType.Exp, scale=float(SCALE)
            )
            return e

        def emit_av_pair(blk, q, e_tiles):
            av = av_tile(f"av{blk}_{q}")
            for a in range(NA):
                nc.tensor.matmul(
                    av[:, 0:512],
                    v_sb[a][:, 256 * q : 256 * q + 128],
                    e_tiles[a][:, 0:512],
                    start=(a == 0), stop=(a == NA - 1),
                )
            for a in range(NA):
                nc.tensor.matmul(
                    av[:, 512:1024],
                    v_sb[a][:, 256 * q + 128 : 256 * (q + 1)],
                    e_tiles[a][:, 512:1024],
                    start=(a == 0), stop=(a == NA - 1),
                )
            rb = p_rb.tile([128, 1024], F32, tag="rb", bufs=2)
            nc.vector.reciprocal_approx_fast(rb[0:D, :], av[0:D, :])
            nc.vector.tensor_mul(
                ot_sb[blk][q][0:D, :], av[D : 2 * D, 0:512], rb[0:D, 0:512]
            )
            nc.vector.tensor_mul(
                ot_sb[blk][q][D : 2 * D, :], av[D : 2 * D, 512:1024], rb[0:D, 512:1024]
            )

        def emit_q_mtile(blk, m):
            ps = mm_tile(f"qp{blk}_{m}")
            for k in range(KD):
                nc.tensor.matmul(
                    ps[:], w_c(wqb_sb, k)[:, 128 * m : 128 * (m + 1)], xt_c(blk, k),
                    start=(k == 0), stop=(k == KD - 1),
                )
            qt = p_qt.tile([128, BLK], BF16, tag="qt", bufs=8)
            nc.vector.tensor_copy(qt[:], ps[:])
            qt_sb[blk][m] = qt

        yt_cur = [None]

        def emit_proj_tile(blk, idx, evac=None):
            """idx = 2*tt + n.  n=0 allocates yt [128,1024]; n=1 DMAs it."""
            tt, n = idx // 2, idx % 2
            ps = mm_tile(f"pj{blk}_{idx}")
            for k2 in range(KQ):
                nc.tensor.matmul(
                    ps[:],
                    ot_sb[blk][k2][:, 128 * tt : 128 * (tt + 1)],
                    wp_sb[:, 1024 * k2 + 512 * n : 1024 * k2 + 512 * (n + 1)],
                    start=(k2 == 0), stop=(k2 == KQ - 1),
                )
            if n == 0:
                yt_cur[0] = p_y.tile(
                    [128, 1024], BF16, tag="y", bufs=4, name=f"yt{blk}_{tt}"
                )
            yt = yt_cur[0]
            if evac is None:
                nc.vector.tensor_copy(yt[:, 512 * n : 512 * (n + 1)], ps[:])
            else:
                nc.scalar.copy(yt[:, 512 * n : 512 * (n + 1)], ps[:])
            if n == 1:
                # alternate queues so y drain runs 2x and never queues
                # behind the sync-queue xt prefetch
                eng = nc.sync if (blk + tt) % 2 == 0 else nc.scalar
                eng.dma_start(
                    y[BLK * blk + 128 * tt : BLK * blk + 128 * (tt + 1), :],
                    yt[:],
                )

        # ---- steady state: scores are software-pipelined ONE STEP AHEAD.
        # Step for pair p runs: AV(p-1), qproj, proj x2, scores(p+1).
        # The Exp of pair p executes during step p with ~4µs slack on every
        # cross-engine edge, so neither the PE FIFO nor ScalarE ever blocks
        # on ps2-slot reuse. ----
        tail_ps = {}

        def tail_mm(pair, tt, n, k2s):
            for k2 in k2s:
                nc.tensor.matmul(
                    pair[n],
                    ot_sb[NBLK - 1][k2][:, 128 * tt : 128 * (tt + 1)],
                    wp_sb[:, 1024 * k2 + 512 * n : 1024 * k2 + 512 * (n + 1)],
                    start=(k2 == 0), stop=(k2 == KQ - 1),
                )

        def tail_evac(tt):
            big, pair = tail_ps[tt]
            yt = p_y.tile([128, 1024], BF16, tag="y", bufs=4, name=f"ytail{tt}")
            yrows = y[
                BLK * (NBLK - 1) + 128 * tt : BLK * (NBLK - 1) + 128 * (tt + 1), :
            ]
            eng = nc.sync if tt % 2 == 0 else nc.scalar
            if big is not None:
                nc.vector.tensor_copy(yt[:, 0:512], big[:, 0:512])
                eng.dma_start(yrows[:, 0:512], yt[:, 0:512])
                nc.vector.tensor_copy(yt[:, 512:1024], big[:, 512:1024])
                eng.dma_start(yrows[:, 512:1024], yt[:, 512:1024])
            else:
                nc.scalar.copy(yt[:, 0:512], pair[0])
                eng.dma_start(yrows[:, 0:512], yt[:, 0:512])
                nc.scalar.copy(yt[:, 512:1024], pair[1])
                eng.dma_start(yrows[:, 512:1024], yt[:, 512:1024])

        # Per-step PE weave: [sc0, AV(p-1), sc1, sc2, qproj, sc3, proj, proj]
        # — the next pair's score chunks are spread between the major ops so
        # every ps2-slot-reuse edge and every exp→AV edge has >=0.7µs slack.
        e_this = [emit_score_tile(0, 0, a) for a in range(NA)]
        av_pend = None
        for blk in range(NBLK):
            if blk + 2 < NBLK:
                issue_xt(blk + 2, nc.sync)
            for q in range(KQ):
                ot_sb[blk][q] = p_ot.tile(
                    [128, BLK], BF16, tag="ot", bufs=8, name=f"ot{blk}_{q}"
                )
            for i in range(NP):
                ops = []
                if av_pend is not None:
                    ops.append(partial(emit_av_pair, *av_pend))
                av_pend = (blk, i, e_this)
                if blk + 1 < NBLK:
                    ops.append(partial(emit_q_mtile, blk + 1, i))
                if blk > 0:
                    ops.append(partial(emit_proj_tile, blk - 1, 2 * i))
                    ops.append(partial(emit_proj_tile, blk - 1, 2 * i + 1))
                if i + 1 < NP:
                    nxt = (blk, i + 1)
                elif blk + 1 < NBLK:
                    nxt = (blk + 1, 0)
                else:
                    nxt = None
                # weave positions: sc0 sc1 op0 op1 sc2 sc3 op2 op3 — score
                # pairs clustered in twos (each full-MM<->half-MM transition
                # costs ~90ns of PE drain, so fewer groups is faster)
                e_next = [None] * NA
                sc = (
                    (lambda a: e_next.__setitem__(a, emit_score_tile(*nxt, a)))
                    if nxt
                    else (lambda a: None)
                )
                sc(0)
                sc(1)
                if len(ops) > 0:
                    ops[0]()
                if len(ops) > 1:
                    ops[1]()
                if blk == 0 and i == 0:
                    emit_v_part2()
                sc(2)
                sc(3)
                for op in ops[2:]:
                    op()
                e_this = e_next if nxt else None

        # ---- tail: AV of the final pair (split reciprocal so ot lands
        # sooner), then out-proj of the last block with psums spread over
        # the now-idle ps2 slots so evacuation never gates the PE ----
        blk_t, q_t, e_t = av_pend
        av = av_tile(f"av{blk_t}_{q_t}")
        for a in range(NA):
            nc.tensor.matmul(
                av[:, 0:512], v_sb[a][:, 256 * q_t : 256 * q_t + 128],
                e_t[a][:, 0:512], start=(a == 0), stop=(a == NA - 1),
            )
        for a in range(NA):
            nc.tensor.matmul(
                av[:, 512:1024], v_sb[a][:, 256 * q_t + 128 : 256 * (q_t + 1)],
                e_t[a][:, 512:1024], start=(a == 0), stop=(a == NA - 1),
            )
        rb = p_rb.tile([128, 1024], F32, tag="rb", bufs=2, name="rbtail")
        ot_t = ot_sb[blk_t][q_t]
        nc.vector.reciprocal_approx_fast(rb[0:D, 0:512], av[0:D, 0:512])
        nc.vector.tensor_mul(ot_t[0:D, :], av[D : 2 * D, 0:512], rb[0:D, 0:512])
        nc.vector.reciprocal_approx_fast(rb[0:D, 512:1024], av[0:D, 512:1024])
        nc.vector.tensor_mul(
            ot_t[D : 2 * D, :], av[D : 2 * D, 512:1024], rb[0:D, 512:1024]
        )
        for tt in range(2):
            big = ps2_tile(f"tailps{tt}")
            tail_ps[tt] = (big, (big[:, 0:512], big[:, 512:1024]))
        tail_ps[2] = (None, (mm_tile("tailp2_0"), mm_tile("tailp2_1")))
        for tt in range(2):
            for n in range(2):
                tail_mm(tail_ps[tt][1], tt, n, (0, 1, 2))
        for tt in range(2):
            for n in range(2):
                tail_mm(tail_ps[tt][1], tt, n, (3,))
            tail_evac(tt)
        for n in range(2):
            tail_mm(tail_ps[2][1], 2, n, (0, 1, 2, 3))
        tail_evac(2)
        tail_ps[3] = (None, (mm_tile("tailp3_0"), mm_tile("tailp3_1")))
        for n in range(2):
            tail_mm(tail_ps[3][1], 3, n, (0, 1, 2, 3))
        tail_evac(3)

    nc.compile()
    return nc


def _shard_inputs(x, Wqkv, Wq, Wproj):
    """Per-core inputs: core i -> (batch i//2, head half i%2)."""
    x = np.asarray(x, dtype=np.float32)
    Wqkv = np.asarray(Wqkv, dtype=np.float32)
    Wq = np.asarray(Wq, dtype=np.float32)
    Wproj = np.asarray(Wproj, dtype=np.float32)

    bf16 = ml_dtypes.bfloat16

    def tile_w(w):  # [K*128, C] -> [128, K*C] with [p, C*k+c] = w[128k+p, c]
        k = w.shape[0] // 128
        return (
            w.reshape(k, 128, w.shape[1]).transpose(1, 0, 2).reshape(128, -1)
        ).astype(bf16)

    halves = []
    for j in range(2):
        hs = slice(HD * j, HD * (j + 1))
        halves.append(
            {
                "wk": tile_w(Wqkv[:, DIM : 2 * DIM][:, hs]),
                "wv": tile_w(Wqkv[:, 2 * DIM :][:, hs]),
                "wqa": tile_w(Wqkv[:, :DIM][:, hs]),
                "wqb": tile_w(Wq[:, hs]),
                "wproj": tile_w(Wproj[hs, :]),
            }
        )
    in_maps = []
    for core in range(N_CORES):
        b, j = core // 2, core % 2
        m = dict(halves[j])
        # [128*blk + p, 512*k + t] = x[b, 512*blk + t, 128*k + p]
        m["xT"] = (
            x[b].reshape(NBLK, BLK, KD, 128).transpose(0, 3, 2, 1).reshape(
                NBLK * 128, KD * BLK
            )
        ).astype(bf16)
        in_maps.append(m)
    return in_maps


def kernel(x, Wqkv, bqkv, Wq, bq, Wproj, bproj, num_anchor_tokens, **run_kwargs):
    assert int(num_anchor_tokens) == A
    if "nc" not in _COMPILED:
        _COMPILED["nc"] = build_kernel()
    nc = _COMPILED["nc"]
    in_maps = _shard_inputs(x, Wqkv, Wq, Wproj)
    res = run_bass_kernel_spmd(
        nc, in_maps, core_ids=list(range(N_CORES)), **run_kwargs
    )
    bproj = np.asarray(bproj, dtype=np.float32)
    out = np.empty((B, S, DIM), dtype=np.float32)
    for b in range(B):
        out[b] = np.asarray(res.results[2 * b]["y"], dtype=np.float32)
        out[b] += np.asarray(res.results[2 * b + 1]["y"], dtype=np.float32)
    out += bproj[None, None, :]
    _COMPILED["last_result"] = res
    return out


# revision 45
# speedup vs baseline: 1.1907x; 1.1907x over previous
"""AnchorAttention Trainium2 kernel, SPMD over 8 NeuronCores — head-split.

Sharding: core i -> (batch b = i//2, head half j = i%2).  Each core
processes ALL 4096 tokens of its batch for its 8 heads: QKV/Q
column-parallel, proj row-parallel; the host sums the two proj partials
per batch and adds bproj.

v2 changes vs v1 (263µs baseline):
  - Scores for each (a-tile, head-pair) land in ONE [128,1024] f32 PSUM
    tile ([pse|pso], 2 banks); a single Exp ACTIVATE covers both heads,
    halving ScalarE instruction count (the v1 steady-state pacer).
  - AV even/odd accumulate into one [128,1024] PSUM tile; one
    reciprocal over [64,1024] covers both denominators.
  - PSUM laid out by tag: ps2 2x2 banks, av 1x2, mm 2x1 = 8 banks, with
    the per-step PE issue order arranged so score-bank reuse never
    waits on the Exp that frees it.
  - All DMA moved to the sync-engine HWDGE queue (gpsimd issues
    nothing), batched: one dma_start per x block [128,8x512], weights
    in halves, y stores as [128,1024] pairs.  This removes v1's 10.8µs
    software-DGE drain + late y-issue serialization in the tail.
"""

import sys
from contextlib import ExitStack
from functools import partial

sys.path.insert(0, "/opt/trn_rl_repo")

import ml_dtypes
import numpy as np

import concourse.bass as bass
import concourse.mybir as mybir
import concourse.tile as tile
from concourse import bacc
from concourse.bass_utils import run_bass_kernel_spmd

F32 = mybir.dt.float32
BF16 = mybir.dt.bfloat16

B, S, DIM = 4, 4096, 1024
H, D = 16, 64
A = 512              # anchor tokens
TOK = 4096           # tokens per core (full batch)
NBLK = 8             # 512-token blocks per core
BLK = 512
N_CORES = 8
SCALE = 1.0 / np.sqrt(D)

HD = DIM // 2        # qk/v dims per core (8 heads x 64)
KQ = HD // 128       # 4 qk-dim tiles per core
KD = DIM // 128      # 8 contraction tiles (x width)
NA = A // 128        # 4 anchor tiles
NP = H // 4          # 4 head pairs per core

_COMPILED = {}


def build_kernel():
    nc = bacc.Bacc(trn_type="TRN2", target_bir_lowering=False)

    # All inputs arrive PRE-TILED by the host so each [128, 4096] SBUF tile
    # is 128 contiguous 8KB DRAM runs — one dma_start per tensor at full
    # descriptor efficiency.  xT row block blk holds x-block blk with layout
    # [p, 512*k + t] = x[512*blk + t, 128*k + p]; weights [p, 512*k + c].
    xT = nc.declare_dram_parameter("xT", [NBLK * 128, KD * BLK], BF16, isOutput=False)
    wk = nc.declare_dram_parameter("wk", [128, KD * BLK], BF16, isOutput=False)
    wv = nc.declare_dram_parameter("wv", [128, KD * BLK], BF16, isOutput=False)
    wqa = nc.declare_dram_parameter("wqa", [128, KD * BLK], BF16, isOutput=False)
    wqb = nc.declare_dram_parameter("wqb", [128, KD * BLK], BF16, isOutput=False)
    wproj = nc.declare_dram_parameter("wproj", [128, KQ * DIM], BF16, isOutput=False)
    y = nc.declare_dram_parameter("y", [TOK, DIM], BF16, isOutput=True)

    with tile.TileContext(nc) as tc, ExitStack() as ctx:
        # one SBUF pool for everything (per-tag bufs) — fewer pools means
        # fewer teardown drains / semaphore resets in the measured epilogue
        sb = ctx.enter_context(tc.tile_pool(name="sb", bufs=1))
        const = p_w = p_kt = p_v = sb
        p_xt = p_qt = p_exp = p_rb = p_ot = p_y = sb
        # PSUM: ps2 2x[128,1024] (4 banks) + av 1x[128,1024] (2 banks)
        #       + mm 2x[128,512] (2 banks) = 8 banks exactly.
        p_ps = ctx.enter_context(tc.tile_pool(name="p_ps", bufs=1, space="PSUM"))

        def ps2_tile(name):
            return p_ps.tile([128, 1024], F32, tag="ps2", bufs=2, name=name)

        def av_tile(name):
            return p_ps.tile([128, 1024], F32, tag="av", bufs=1, name=name)

        def mm_tile(name):
            return p_ps.tile([128, BLK], F32, tag="mm", bufs=2, name=name)

        # ---- PE warm-up while the first DMAs land.  Sized to end right as
        # xt0/wk finish landing: too long delays KT, too short lets the HAM
        # MID window re-throttle the PE before KT. ----
        warm = const.tile([128, 512], BF16, tag="warm")
        nc.vector.memset(warm[:], 0.0)
        wps = av_tile("warmps")
        NWARM = 16
        for i in range(NWARM):
            nc.tensor.matmul(
                wps[:, 0:512], warm[:, 0:128], warm[:],
                start=(i == 0), stop=(i == NWARM - 1),
            )

        # ---- input DMAs: one issue per tensor (128 x 8KB descriptors).
        # The anchor tokens ARE the first x block, so xt0 doubles as aT. ----
        xt_sb = [None] * NBLK

        def issue_xt(blk, eng):
            t = p_xt.tile([128, KD * BLK], BF16, tag="xt", bufs=3, name=f"xt{blk}")
            eng.dma_start(t[:], xT[128 * blk : 128 * (blk + 1), :])
            xt_sb[blk] = t

        def load_w(dram, name, eng):
            t = p_w.tile([128, dram.shape[1]], BF16, tag=name, name=name)
            eng.dma_start(t[:], dram[:, :])
            return t

        # xt0+wk are the KT-critical 2MB: split each across BOTH queues so
        # their first halves land ~5µs before the second halves; KT runs in
        # two k-half passes paced to that.
        t0 = p_xt.tile([128, KD * BLK], BF16, tag="xt", bufs=3, name="xt0")
        HB = KD * BLK // 2
        nc.sync.dma_start(t0[:, 0:HB], xT[0:128, 0:HB])
        wk_sb = p_w.tile([128, KD * BLK], BF16, tag="wk", name="wk_t")
        nc.scalar.dma_start(wk_sb[:, 0:HB], wk[:, 0:HB])
        nc.scalar.dma_start(t0[:, HB:], xT[0:128, HB:])
        nc.sync.dma_start(wk_sb[:, HB:], wk[:, HB:])
        xt_sb[0] = t0
        wqa_sb = load_w(wqa, "wqa", nc.scalar)
        wv_sb = load_w(wv, "wv", nc.sync)
        issue_xt(1, nc.sync)
        wqb_sb = load_w(wqb, "wqb", nc.scalar)
        wp_sb = load_w(wproj, "wp", nc.scalar)               # [128, 4*1024]

        def xt_c(blk, k):  # [128,512] chunk k of block blk
            return xt_sb[blk][:, BLK * k : BLK * (k + 1)]

        def w_c(t, k):     # [128,512] chunk k of a packed qkv weight
            return t[:, BLK * k : BLK * (k + 1)]

        # ---- KT[qk, a] = Wk^T aT, in two k-half passes paced to the split
        # DMAs (first pass starts as soon as the first halves land) ----
        ktps = [ps2_tile(f"ktps{h}") for h in range(2)]
        for half in range(2):
            for m in range(KQ):
                for k in range(KD // 2 * half, KD // 2 * (half + 1)):
                    nc.tensor.matmul(
                        ktps[m // 2][:, 512 * (m % 2) : 512 * (m % 2) + 512],
                        w_c(wk_sb, k)[:, 128 * m : 128 * (m + 1)],
                        xt_c(0, k),
                        start=(k == 0), stop=(k == KD - 1),
                    )
            if half == 0:
                # cushion: keep the PE busy while the second DMA halves
                # land, so the HAM stays at full clock for KT-B/V
                for i in range(4):
                    nc.tensor.matmul(
                        wps[:, 0:512], warm[:, 0:128], warm[:],
                        start=(i == 0), stop=(i == 3),
                    )
        # packed kt: kt_sb[h] = [kt(2h) | kt(2h+1)], each [128, A]
        kt_sb = []
        for h in range(2):
            kt = p_kt.tile([128, 2 * A], BF16, name=f"kt{h}", tag=f"kt{h}")
            nc.vector.tensor_copy(kt[:], ktps[h][:])
            kt_sb.append(kt)

        def kt_c(i):       # [128, A] view for head pair i
            return kt_sb[i // 2][:, A * (i % 2) : A * (i % 2) + A]

        # ---- V (a-major), then packed [ones|V_even|ones|V_odd] tiles ----
        for i in range(2):  # cushion over the wv-DMA wait
            nc.tensor.matmul(
                wps[:, 0:512], warm[:, 0:128], warm[:], start=(i == 0), stop=(i == 1)
            )
        vps = [ps2_tile(f"vps{h}") for h in range(2)]
        for a in range(NA):
            for k in range(KD):
                nc.tensor.matmul(
                    vps[a // 2][:, 512 * (a % 2) : 512 * (a % 2) + 512],
                    xt_c(0, k)[:, 128 * a : 128 * (a + 1)],
                    w_c(wv_sb, k),
                    start=(k == 0), stop=(k == KD - 1),
                )
        v_sb = []
        for a in range(NA):
            t = p_v.tile([128, 2 * HD], BF16, name=f"v{a}", tag=f"v{a}")
            nc.vector.memset(
                t[:].rearrange("p (hp c) -> p hp c", c=2 * D)[:, :, 0:D], 1.0
            )
            v_sb.append(t)
        for a in range(NA):
            vr = v_sb[a][:].rearrange("p (hp c) -> p hp c", c=4 * D)
            pr = vps[a // 2][:, 512 * (a % 2) : 512 * (a % 2) + 512].rearrange(
                "p (hp c) -> p hp c", c=2 * D
            )
            nc.vector.tensor_copy(vr[:, :, D : 2 * D], pr[:, :, 0:D])
            nc.vector.tensor_copy(vr[:, :, 3 * D : 4 * D], pr[:, :, D : 2 * D])

        # ---- Q-proj of block 0 (anchors: wqa), m-major ----
        qt_sb = [[None] * KQ for _ in range(NBLK)]
        qps = [ps2_tile(f"q0ps{h}") for h in range(2)]
        for m in range(KQ):
            for k in range(KD):
                nc.tensor.matmul(
                    qps[m // 2][:, 512 * (m % 2) : 512 * (m % 2) + 512],
                    w_c(wqa_sb, k)[:, 128 * m : 128 * (m + 1)],
                    xt_c(0, k),
                    start=(k == 0), stop=(k == KD - 1),
                )
        for m in range(KQ):
            qt = p_qt.tile([128, BLK], BF16, tag="qt", bufs=8, name=f"q0_{m}")
            nc.vector.tensor_copy(
                qt[:], qps[m // 2][:, 512 * (m % 2) : 512 * (m % 2) + 512]
            )
            qt_sb[0][m] = qt

        ot_sb = [[None] * KQ for _ in range(NBLK)]

        def emit_score_tile(blk, i, a):
            """One [128,1024] PSUM tile = [pse(a)|pso(a)] for head pair i;
            one Exp ACTIVATE over both halves -> e tile [128,1024] bf16."""
            ps = ps2_tile(f"sc{blk}_{i}_{a}")
            nc.tensor.matmul(
                ps[:, 0:512],
                kt_c(i)[0:D, 128 * a : 128 * (a + 1)],
                qt_sb[blk][i][0:D, :],
                start=True, stop=True,
                tile_position=(0, 0),
            )
            nc.tensor.matmul(
                ps[:, 512:1024],
                kt_c(i)[D : 2 * D, 128 * a : 128 * (a + 1)],
                qt_sb[blk][i][D : 2 * D, :],
                start=True, stop=True,
                tile_position=(64, 0),
            )
            e = p_exp.tile([128, 1024], BF16, tag="exp", bufs=12)
            nc.scalar.activation(
                e[:], ps[:], mybir.ActivationFunctionType.Exp, scale=float(SCALE)
            )
            return e

        def emit_av_pair(blk, q, e_tiles):
            av = av_tile(f"av{blk}_{q}")
            for a in range(NA):
                nc.tensor.matmul(
                    av[:, 0:512],
                    v_sb[a][:, 256 * q : 256 * q + 128],
                    e_tiles[a][:, 0:512],
                    start=(a == 0), stop=(a == NA - 1),
                )
            for a in range(NA):
                nc.tensor.matmul(
                    av[:, 512:1024],
                    v_sb[a][:, 256 * q + 128 : 256 * (q + 1)],
                    e_tiles[a][:, 512:1024],
                    start=(a == 0), stop=(a == NA - 1),
                )
            rb = p_rb.tile([128, 1024], F32, tag="rb", bufs=2)
            nc.vector.reciprocal_approx_fast(rb[0:D, :], av[0:D, :])
            nc.vector.tensor_mul(
                ot_sb[blk][q][0:D, :], av[D : 2 * D, 0:512], rb[0:D, 0:512]
            )
            nc.vector.tensor_mul(
                ot_sb[blk][q][D : 2 * D, :], av[D : 2 * D, 512:1024], rb[0:D, 512:1024]
            )

        def emit_q_mtile(blk, m):
            ps = mm_tile(f"qp{blk}_{m}")
            for k in range(KD):
                nc.tensor.matmul(
                    ps[:], w_c(wqb_sb, k)[:, 128 * m : 128 * (m + 1)], xt_c(blk, k),
                    start=(k == 0), stop=(k == KD - 1),
                )
            qt = p_qt.tile([128, BLK], BF16, tag="qt", bufs=8)
            nc.vector.tensor_copy(qt[:], ps[:])
            qt_sb[blk][m] = qt

        yt_cur = [None]

        def emit_proj_tile(blk, idx, evac=None):
            """idx = 2*tt + n.  n=0 allocates yt [128,1024]; n=1 DMAs it."""
            tt, n = idx // 2, idx % 2
            ps = mm_tile(f"pj{blk}_{idx}")
            for k2 in range(KQ):
                nc.tensor.matmul(
                    ps[:],
                    ot_sb[blk][k2][:, 128 * tt : 128 * (tt + 1)],
                    wp_sb[:, 1024 * k2 + 512 * n : 1024 * k2 + 512 * (n + 1)],
                    start=(k2 == 0), stop=(k2 == KQ - 1),
                )
            if n == 0:
                yt_cur[0] = p_y.tile(
                    [128, 1024], BF16, tag="y", bufs=4, name=f"yt{blk}_{tt}"
                )
            yt = yt_cur[0]
            if evac is None:
                nc.vector.tensor_copy(yt[:, 512 * n : 512 * (n + 1)], ps[:])
            else:
                nc.scalar.copy(yt[:, 512 * n : 512 * (n + 1)], ps[:])
            if n == 1:
                # alternate queues so y drain runs 2x and never queues
                # behind the sync-queue xt prefetch
                eng = nc.sync if (blk + tt) % 2 == 0 else nc.scalar
                eng.dma_start(
                    y[BLK * blk + 128 * tt : BLK * blk + 128 * (tt + 1), :],
                    yt[:],
                )

        # ---- steady state: scores are software-pipelined ONE STEP AHEAD.
        # Step for pair p runs: AV(p-1), qproj, proj x2, scores(p+1).
        # The Exp of pair p executes during step p with ~4µs slack on every
        # cross-engine edge, so neither the PE FIFO nor ScalarE ever blocks
        # on ps2-slot reuse. ----
        tail_ps = {}

        def tail_mm(pair, tt, n, k2s):
            for k2 in k2s:
                nc.tensor.matmul(
                    pair[n],
                    ot_sb[NBLK - 1][k2][:, 128 * tt : 128 * (tt + 1)],
                    wp_sb[:, 1024 * k2 + 512 * n : 1024 * k2 + 512 * (n + 1)],
                    start=(k2 == 0), stop=(k2 == KQ - 1),
                )

        def tail_evac(tt):
            big, pair = tail_ps[tt]
            yt = p_y.tile([128, 1024], BF16, tag="y", bufs=4, name=f"ytail{tt}")
            yrows = y[
                BLK * (NBLK - 1) + 128 * tt : BLK * (NBLK - 1) + 128 * (tt + 1), :
            ]
            eng = nc.sync if tt % 2 == 0 else nc.scalar
            if big is not None:
                nc.vector.tensor_copy(yt[:, 0:512], big[:, 0:512])
                eng.dma_start(yrows[:, 0:512], yt[:, 0:512])
                nc.vector.tensor_copy(yt[:, 512:1024], big[:, 512:1024])
                eng.dma_start(yrows[:, 512:1024], yt[:, 512:1024])
            else:
                nc.scalar.copy(yt[:, 0:512], pair[0])
                eng.dma_start(yrows[:, 0:512], yt[:, 0:512])
                nc.scalar.copy(yt[:, 512:1024], pair[1])
                eng.dma_start(yrows[:, 512:1024], yt[:, 512:1024])

        # Per-step PE weave: [sc0, AV(p-1), sc1, sc2, qproj, sc3, proj, proj]
        # — the next pair's score chunks are spread between the major ops so
        # every ps2-slot-reuse edge and every exp→AV edge has >=0.7µs slack.
        e_this = [emit_score_tile(0, 0, a) for a in range(NA)]
        av_pend = None
        for blk in range(NBLK):
            if blk + 2 < NBLK:
                issue_xt(blk + 2, nc.sync)
            for q in range(KQ):
                ot_sb[blk][q] = p_ot.tile(
                    [128, BLK], BF16, tag="ot", bufs=8, name=f"ot{blk}_{q}"
                )
            for i in range(NP):
                ops = []
                if av_pend is not None:
                    ops.append(partial(emit_av_pair, *av_pend))
                av_pend = (blk, i, e_this)
                if blk + 1 < NBLK:
                    ops.append(partial(emit_q_mtile, blk + 1, i))
                if blk > 0:
                    ops.append(partial(emit_proj_tile, blk - 1, 2 * i))
                    ops.append(partial(emit_proj_tile, blk - 1, 2 * i + 1))
                if i + 1 < NP:
                    nxt = (blk, i + 1)
                elif blk + 1 < NBLK:
                    nxt = (blk + 1, 0)
                else:
                    nxt = None
                # weave positions: sc0 sc1 op0 op1 sc2 sc3 op2 op3 — score
                # pairs clustered in twos (each full-MM<->half-MM transition
                # costs ~90ns of PE drain, so fewer groups is faster)
                e_next = [None] * NA
                sc = (
                    (lambda a: e_next.__setitem__(a, emit_score_tile(*nxt, a)))
                    if nxt
                    else (lambda a: None)
                )
                sc(0)
                sc(1)
                if len(ops) > 0:
                    ops[0]()
                if len(ops) > 1:
                    ops[1]()
                sc(2)
                sc(3)
                for op in ops[2:]:
                    op()
                e_this = e_next if nxt else None

        # ---- tail: AV of the final pair (split reciprocal so ot lands
        # sooner), then out-proj of the last block with psums spread over
        # the now-idle ps2 slots so evacuation never gates the PE ----
        blk_t, q_t, e_t = av_pend
        av = av_tile(f"av{blk_t}_{q_t}")
        for a in range(NA):
            nc.tensor.matmul(
                av[:, 0:512], v_sb[a][:, 256 * q_t : 256 * q_t + 128],
                e_t[a][:, 0:512], start=(a == 0), stop=(a == NA - 1),
            )
        for a in range(NA):
            nc.tensor.matmul(
                av[:, 512:1024], v_sb[a][:, 256 * q_t + 128 : 256 * (q_t + 1)],
                e_t[a][:, 512:1024], start=(a == 0), stop=(a == NA - 1),
            )
        rb = p_rb.tile([128, 1024], F32, tag="rb", bufs=2, name="rbtail")
        ot_t = ot_sb[blk_t][q_t]
        nc.vector.reciprocal_approx_fast(rb[0:D, 0:512], av[0:D, 0:512])
        nc.vector.tensor_mul(ot_t[0:D, :], av[D : 2 * D, 0:512], rb[0:D, 0:512])
        nc.vector.reciprocal_approx_fast(rb[0:D, 512:1024], av[0:D, 512:1024])
        nc.vector.tensor_mul(
            ot_t[D : 2 * D, :], av[D : 2 * D, 512:1024], rb[0:D, 512:1024]
        )
        for tt in range(2):
            big = ps2_tile(f"tailps{tt}")
            tail_ps[tt] = (big, (big[:, 0:512], big[:, 512:1024]))
        tail_ps[2] = (None, (mm_tile("tailp2_0"), mm_tile("tailp2_1")))
        for tt in range(2):
            for n in range(2):
                tail_mm(tail_ps[tt][1], tt, n, (0, 1, 2))
        for tt in range(2):
            for n in range(2):
                tail_mm(tail_ps[tt][1], tt, n, (3,))
            tail_evac(tt)
        for n in range(2):
            tail_mm(tail_ps[2][1], 2, n, (0, 1, 2, 3))
        tail_evac(2)
        tail_ps[3] = (None, (mm_tile("tailp3_0"), mm_tile("tailp3_1")))
        for n in range(2):
            tail_mm(tail_ps[3][1], 3, n, (0, 1, 2, 3))
        tail_evac(3)

    nc.compile()
    return nc


def _shard_inputs(x, Wqkv, Wq, Wproj):
    """Per-core inputs: core i -> (batch i//2, head half i%2)."""
    x = np.asarray(x, dtype=np.float32)
    Wqkv = np.asarray(Wqkv, dtype=np.float32)
    Wq = np.asarray(Wq, dtype=np.float32)
    Wproj = np.asarray(Wproj, dtype=np.float32)

    bf16 = ml_dtypes.bfloat16

    def tile_w(w):  # [K*128, C] -> [128, K*C] with [p, C*k+c] = w[128k+p, c]
        k = w.shape[0] // 128
        return (
            w.reshape(k, 128, w.shape[1]).transpose(1, 0, 2).reshape(128, -1)
        ).astype(bf16)

    halves = []
    for j in range(2):
        hs = slice(HD * j, HD * (j + 1))
        halves.append(
            {
                "wk": tile_w(Wqkv[:, DIM : 2 * DIM][:, hs]),
                "wv": tile_w(Wqkv[:, 2 * DIM :][:, hs]),
                "wqa": tile_w(Wqkv[:, :DIM][:, hs]),
                "wqb": tile_w(Wq[:, hs]),
                "wproj": tile_w(Wproj[hs, :]),
            }
        )
    in_maps = []
    for core in range(N_CORES):
        b, j = core // 2, core % 2
        m = dict(halves[j])
        # [128*blk + p, 512*k + t] = x[b, 512*blk + t, 128*k + p]
        m["xT"] = (
            x[b].reshape(NBLK, BLK, KD, 128).transpose(0, 3, 2, 1).reshape(
                NBLK * 128, KD * BLK
            )
        ).astype(bf16)
        in_maps.append(m)
    return in_maps


def kernel(x, Wqkv, bqkv, Wq, bq, Wproj, bproj, num_anchor_tokens, **run_kwargs):
    assert int(num_anchor_tokens) == A
    if "nc" not in _COMPILED:
        _COMPILED["nc"] = build_kernel()
    nc = _COMPILED["nc"]
    in_maps = _shard_inputs(x, Wqkv, Wq, Wproj)
    res = run_bass_kernel_spmd(
        nc, in_maps, core_ids=list(range(N_CORES)), **run_kwargs
    )
    bproj = np.asarray(bproj, dtype=np.float32)
    out = np.empty((B, S, DIM), dtype=np.float32)
    for b in range(B):
        out[b] = np.asarray(res.results[2 * b]["y"], dtype=np.float32)
        out[b] += np.asarray(res.results[2 * b + 1]["y"], dtype=np.float32)
    out += bproj[None, None, :]
    _COMPILED["last_result"] = res
    return out
